# revision 21
# baseline (speedup 1.0000x reference)
"""Multi-head attention (b=2, sq=skv=2048, dim=1024, 16 heads x 64) on 8 TRN2
NeuronCores.

Sharding: 2 heads per core (head-parallel across batch*heads), with the
matching tensor-parallel column slice of W_qkv and row slice of W_out.  Each
core computes a partial output projection over its 128 head-dims; the
all-reduce of the 8 partials (+ bias) happens on the host during unshard.

v3 design (ACT-bound steady state ~1us/step, PE/DVE trimmed to match):
  - scores: per (qt, j) one 2-bank PSUM tile [128 k, 2 h, 512 q]; the two
    heads' score matmuls (K=64) run concurrently in PE row-halves.
  - exp: one ACTIVATE per step over both heads (N=1024, scale 1/8 fused).
  - PV: col-tiled concurrent pair -- h0 -> acc[0:64], h1 -> acc[64:128]
    (tile_position (0,0)/(0,64) auto-derived), one PSUM bank per q-tile.
  - denominator: running sums of the exp tiles (13 adds/qt on DVE in bf16
    2x mode + 3 on the otherwise-idle Pool engine); at q-tile end two M=1
    ones-matmuls reduce them over k into two PSUM rows, which bounce
    through DRAM to repack [2,512] -> [128,8] so the DVE reciprocal runs
    wide (the 1-lane reciprocal costs 6.5ns/elem -- 3.3us/row), bounce
    back as bf16, and two concurrent K=1 outer-product matmuls broadcast
    r to the 64 partitions of each head.
  - normalization is fused into the acc PSUM->SBUF copy (tensor_mul).
  - v goes through the transposed projection (weight-stationary, N=512
    matmuls) + PE transposes into natural [k-token, dim] layout; the
    x-stationary variant costs ~220ns per N=128 matmul (LDWEIGHTS-bound)
    and is 2x more PE time.
  - emission: only chunk-0 projections precede attention(0); every other
    projection piece and outproj tile is drip-fed through per-step hooks
    (pre-hooks gate scores/PV inputs LOOKAHEAD steps early).  First
    chunks are loaded in ko-halves so projections start earlier; weights
    ride the scalar queue; kv chunks + flush bounces ride sync; q chunks
    + outputs ride gpsimd.
"""

import os
import sys

for _p in ("/opt/trn_rl_repo", "/root/.axon_site/_ro/trn_rl_repo"):
    if os.path.isdir(_p) and _p not in sys.path:
        sys.path.append(_p)

import ml_dtypes
import numpy as np

import concourse.bass as bass  # noqa: F401
import concourse.tile as tile
from concourse import bacc, mybir
from concourse.bass_utils import run_bass_kernel_spmd
from concourse.masks import make_identity

B, SQ, SKV, DIM = 2, 2048, 2048, 1024
HEADS, DH = 16, 64
N_CORES = 8
HPC = HEADS // N_CORES  # heads per core = 2
HD = HPC * DH  # 128 head-dim rows per core
TOK = B * SQ  # 4096
KO = DIM // 128  # 8 contraction chunks of 128
SCALE = DH**-0.5

BF16 = mybir.dt.bfloat16
F32 = mybir.dt.float32

PCHUNK = 512  # token chunk in projections (contiguous per-chunk dram layout)
QTILE = 512  # q tile in attention
KTILE = 128  # k tile (scores psum partition dim)
NKT = SKV // KTILE  # 16
NQT = SQ // QTILE  # 4
NCPB = SQ // PCHUNK  # chunks per batch = 4

BF = ml_dtypes.bfloat16
Exp = mybir.ActivationFunctionType.Exp

LOOKAHEAD = 6


def build():
    nc = bacc.Bacc(
        "TRN2", target_bir_lowering=False, debug=False, num_devices=N_CORES
    )

    NCH = TOK // PCHUNK
    xqt_d = nc.dram_tensor("xqt", [NCH, 128, KO, PCHUNK], BF16, kind="ExternalInput")
    xkvt_d = nc.dram_tensor("xkvt", [NCH, 128, KO, PCHUNK], BF16, kind="ExternalInput")
    wq_d = nc.dram_tensor("wq", [128, KO, HD], BF16, kind="ExternalInput")
    wk_d = nc.dram_tensor("wk", [128, KO, HD], BF16, kind="ExternalInput")
    wv_d = nc.dram_tensor("wv", [128, KO, HD], BF16, kind="ExternalInput")
    wout_d = nc.dram_tensor("wout", [HD, DIM], BF16, kind="ExternalInput")
    out_d = nc.dram_tensor("out", [TOK, DIM], BF16, kind="ExternalOutput")

    xqt = xqt_d.ap()
    xkvt = xkvt_d.ap()

    with tile.TileContext(nc) as tc:
        with (
            tc.tile_pool(name="persist", bufs=1) as persist,
            tc.tile_pool(name="xin", bufs=6) as xin,
            tc.tile_pool(name="exps", bufs=8) as exps,
            tc.tile_pool(name="spool", bufs=2) as spool,
            tc.tile_pool(name="usbp", bufs=3) as usbp,
            tc.tile_pool(name="rbp", bufs=2) as rbp,
            tc.tile_pool(name="bcp", bufs=2) as bcp,
            tc.tile_pool(name="dpkp", bufs=4) as dpkp,
            tc.tile_pool(name="dsp", bufs=2) as dsp,
            tc.tile_pool(name="ost", bufs=5) as ost,
            tc.tile_pool(name="spsum", bufs=2, space="PSUM") as spsum,
            tc.tile_pool(name="accp", bufs=2, space="PSUM") as accp,
            tc.tile_pool(name="miscp", bufs=2, space="PSUM") as miscp,
            tc.tile_pool(name="drp", bufs=2, space="DRAM") as drp,
        ):
            # --- weights on the scalar queue (its own DMA ring) ---
            wk_sb = persist.tile([128, KO, HD], BF16, tag="wk")
            nc.scalar.dma_start(wk_sb[:], wk_d.ap())
            wq_sb = persist.tile([128, KO, HD], BF16, tag="wq")
            nc.scalar.dma_start(wq_sb[:], wq_d.ap())
            wv_sb = persist.tile([128, KO, HD], BF16, tag="wv")
            nc.scalar.dma_start(wv_sb[:], wv_d.ap())
            wout_sb = persist.tile([HD, DIM], BF16, tag="wout")
            nc.scalar.dma_start(wout_sb[:], wout_d.ap())

            ones_col = persist.tile([128, 1], BF16, tag="ones_col")
            nc.vector.memset(ones_col[:], 1.0)
            ones_row = persist.tile([1, DH], BF16, tag="ones_row")
            nc.vector.memset(ones_row[:], 1.0)
            ident = persist.tile([128, DH], BF16, tag="ident")
            make_identity(nc, ident[0:DH, :])
            make_identity(nc, ident[DH : 2 * DH, :])
            # prefetch the exp table set during the initial DMAs
            dummy = persist.tile([1, 8], F32, tag="dummy")
            nc.vector.memset(dummy[:], 0.0)
            nc.scalar.activation(dummy[:], dummy[:], Exp)

            qt_sb, kt_sb, vt_sb, vnat, outT = {}, {}, {}, {}, {}
            for b in range(B):
                qt_sb[b] = persist.tile([HD, SQ], BF16, tag=f"qt{b}", name=f"qt{b}")
                kt_sb[b] = persist.tile([HD, SKV], BF16, tag=f"kt{b}", name=f"kt{b}")
                vt_sb[b] = persist.tile([HD, SKV], BF16, tag=f"vt{b}", name=f"vt{b}")
                vnat[b] = persist.tile(
                    [128, NKT, HD], BF16, tag=f"vn{b}", name=f"vn{b}"
                )
                outT[b] = persist.tile([HD, SQ], BF16, tag=f"ot{b}", name=f"ot{b}")

            # ---------- projection pieces ----------
            kv_tiles = {0: {}, 1: {}}
            q_tiles = {0: {}, 1: {}}
            projps = {}

            def _load(dst, x_ap, b, tt):
                # halves ride both DMA rings so no single transfer
                # exceeds ~5us and the rings stay load-balanced
                def go():
                    xt = xin.tile([128, KO, PCHUNK], BF16, tag="x")
                    src = x_ap[b * NCPB + tt]
                    nc.sync.dma_start(xt[:, 0:4, :], src[:, 0:4, :])
                    nc.gpsimd.dma_start(xt[:, 4:8, :], src[:, 4:8, :])
                    dst[b][tt] = xt

                return go

            def kv_load(b, tt):
                return _load(kv_tiles, xkvt, b, tt)

            def q_load(b, tt):
                return _load(q_tiles, xqt, b, tt)

            def proj_half(dst_d, w_sb, src_d, b, tt, half):
                """4 of the 8 ko-accumulation matmuls; copy on second half."""

                def go():
                    if half == 0:
                        projps[0] = miscp.tile(
                            [128, PCHUNK], F32, tag="m", name="projp"
                        )
                    ps = projps[0]
                    xt = src_d[b][tt]
                    for ko in range(half * 4, half * 4 + 4):
                        nc.tensor.matmul(
                            ps[:],
                            w_sb[:, ko, :],
                            xt[:, ko, :],
                            start=(ko == 0),
                            stop=(ko == KO - 1),
                        )
                    if half == 1:
                        t0 = tt * PCHUNK
                        nc.vector.tensor_copy(
                            dst_d[b][:, t0 : t0 + PCHUNK], ps[:]
                        )

                return go

            def k_half(b, tt, half):
                return proj_half(kt_sb, wk_sb, kv_tiles, b, tt, half)

            def q_half(b, tt, half):
                return proj_half(qt_sb, wq_sb, q_tiles, b, tt, half)

            def v_half(b, tt, half):
                return proj_half(vt_sb, wv_sb, kv_tiles, b, tt, half)

            def v_group(b, jg, h):
                """PE-transpose k-tiles 4jg..4jg+3 of head h of vT into
                natural [k-token, dim] layout."""

                def go():
                    h_sl = slice(h * DH, (h + 1) * DH)
                    tp = miscp.tile([128, 4, DH], BF16, tag="m", name="vtp")
                    for i in range(4):
                        j = jg * 4 + i
                        nc.tensor.transpose(
                            tp[:, i, :],
                            vt_sb[b][h_sl, j * KTILE : (j + 1) * KTILE],
                            ident[h_sl, :],
                        )
                    nc.vector.tensor_copy(
                        vnat[b][:, jg * 4 : (jg + 1) * 4, h_sl], tp[:]
                    )

                return go

            # ---------- flush (denominator + normalization) ----------
            # Four stages scheduled ~steps apart via hooks so the in-order
            # PE/DVE queues never wait on the DRAM repack round-trip.
            def flush_a(b, qt, acc, S):
                """j==15: reduce S over k, stage everything out of PSUM,
                kick off the repack DMAs.  Frees acc + dsum immediately."""
                dsum = miscp.tile([128, QTILE], F32, tag="m", name="dsum")
                for h in range(HPC):
                    nc.tensor.matmul(
                        dsum[h * 32 : h * 32 + 1, :],
                        ones_col[:],
                        S[:, h, :],
                        start=True,
                        stop=True,
                        skip_group_check=True,
                    )
                dstage = dsp.tile([33, QTILE], F32, tag="ds", name="dstage")
                for h in range(HPC):
                    nc.vector.tensor_copy(
                        dstage[h * 32 : h * 32 + 1, :],
                        dsum[h * 32 : h * 32 + 1, :],
                    )
                # unnormalized accumulator out of PSUM (bf16; the later
                # normalizing multiply runs SBUF-only in 2x mode)
                usb = usbp.tile([128, QTILE], BF16, tag="u", name="usb")
                nc.vector.tensor_copy(usb[:], acc[:])
                d1 = drp.tile([HPC, QTILE], F32, tag="d1", name="d1")
                for h in range(HPC):
                    nc.sync.dma_start(
                        d1[h : h + 1, :], dstage[h * 32 : h * 32 + 1, :]
                    )
                dpk = dpkp.tile([128, HPC, 4], F32, tag="dp", name="dpk")
                for h in range(HPC):
                    nc.sync.dma_start(
                        dpk[:, h, :],
                        d1[h, :].rearrange("(p f) -> p f", p=128),
                    )
                return usb, dpk

            pending = {}

            def fb(b, qt):
                """~2 q-tiles later: dpk has landed; wide reciprocal, send
                the bf16 reciprocals back to a [1, 2, 512] row."""

                def go():
                    st = pending[(b, qt)]
                    dpk = st[3]
                    rpk = dpkp.tile([128, HPC, 4], BF16, tag="rp", name="rpk")
                    with nc.allow_low_precision(reason="bf16 softmax recip"):
                        nc.vector.reciprocal(rpk[:], dpk[:])
                    d2 = drp.tile([HPC, QTILE], BF16, tag="d2", name="d2")
                    for h in range(HPC):
                        nc.sync.dma_start(
                            d2[h, :].rearrange("(p f) -> p f", p=128),
                            rpk[:, h, :],
                        )
                    rb = rbp.tile([1, HPC, QTILE], BF16, tag="rb", name="rb")
                    nc.sync.dma_start(rb[0:1, :, :], d2[:])
                    st.append(rb)

                return go

            def fcd(b, qt):
                """broadcast r to the 64 partitions of each head (K=1
                outer products, concurrent col halves), then the
                normalizing multiply into outT (all-SBUF, 2x mode)."""

                def go():
                    st = pending.pop((b, qt))
                    _b, _qt, usb, _dpk, rb = st
                    q_sl = slice(qt * QTILE, (qt + 1) * QTILE)
                    bc = miscp.tile([128, QTILE], F32, tag="m", name="bc")
                    for h in range(HPC):
                        nc.tensor.matmul(
                            bc[h * DH : (h + 1) * DH, :],
                            ones_row[:],
                            rb[0:1, h, :],
                            start=True,
                            stop=True,
                            skip_group_check=True,
                        )
                    bcs = bcp.tile([128, QTILE], BF16, tag="bc", name="bcs")
                    nc.vector.tensor_copy(bcs[:], bc[:])
                    for h in range(HPC):
                        h_sl = slice(h * DH, (h + 1) * DH)
                        nc.vector.tensor_mul(
                            outT[b][h_sl, q_sl], usb[h_sl, :], bcs[h_sl, :]
                        )

                return go

            # ---------- output projection ----------
            def outproj(b, tt, on_scalar=False):
                def go():
                    t_sl = slice(tt * 128, (tt + 1) * 128)
                    ob = ost.tile([128, 2, 512], BF16, tag="o")
                    for nt in range(DIM // 512):
                        ps = miscp.tile([128, PCHUNK], F32, tag="m", name="projo")
                        nc.tensor.matmul(
                            ps[:],
                            outT[b][:, t_sl],
                            wout_sb[:, nt * 512 : (nt + 1) * 512],
                            start=True,
                            stop=True,
                        )
                        if on_scalar:
                            nc.scalar.copy(ob[:, nt, :], ps[:])
                        else:
                            nc.vector.tensor_copy(ob[:, nt, :], ps[:])
                    dq = nc.scalar if on_scalar else nc.gpsimd
                    dq.dma_start(
                        out_d.ap()[
                            b * SQ + tt * 128 : b * SQ + (tt + 1) * 128, :
                        ].rearrange("t (n c) -> t n c", n=2),
                        ob[:],
                    )

                return go

            # ---------- attention ----------
            def attention(b, pre_hooks, post_hooks):
                NT = NQT * NKT
                sps, st, accs = {}, {}, {}

                def emit_scores(t):
                    qt, j = divmod(t, NKT)
                    q_sl = slice(qt * QTILE, (qt + 1) * QTILE)
                    k_sl = slice(j * KTILE, (j + 1) * KTILE)
                    sp = spsum.tile([128, HPC, QTILE], F32, tag="s", name="sp")
                    sps[t] = sp
                    for h in range(HPC):
                        h_sl = slice(h * DH, (h + 1) * DH)
                        nc.tensor.matmul(
                            sp[:, h, :],
                            kt_sb[b][h_sl, k_sl],
                            qt_sb[b][h_sl, q_sl],
                            start=True,
                            stop=True,
                        )

                def emit_tail(t):
                    qt, j = divmod(t, NKT)
                    sp = sps.pop(t)
                    ex = exps.tile([128, HPC, QTILE], BF16, tag="e", name="ex")
                    nc.scalar.activation(ex[:], sp[:], Exp, scale=SCALE)
                    if j == 0:
                        accs[qt] = accp.tile(
                            [128, QTILE], F32, tag="acc", name="acc"
                        )
                        st[qt] = spool.tile(
                            [128, HPC, QTILE], BF16, tag="S", name="S"
                        )
                        nc.vector.tensor_copy(st[qt][:], ex[:])
                    else:
                        nc.vector.tensor_add(st[qt][:], st[qt][:], ex[:])
                    for h in range(HPC):
                        nc.tensor.matmul(
                            accs[qt][h * DH : (h + 1) * DH, :],
                            vnat[b][:, j, h * DH : (h + 1) * DH],
                            ex[:, h, :],
                            start=(j == 0),
                            stop=(j == NKT - 1),
                            skip_group_check=True,
                        )
                    if j == NKT - 1:
                        usb, dpk = flush_a(b, qt, accs.pop(qt), st.pop(qt))
                        pending[(b, qt)] = [b, qt, usb, dpk]
                    for fn in post_hooks.get((qt, j), []):
                        fn()

                for t in range(NT + LOOKAHEAD):
                    if t < NT:
                        for fn in pre_hooks.get(t, []):
                            fn()
                        emit_scores(t)
                    if t >= LOOKAHEAD:
                        emit_tail(t - LOOKAHEAD)

            # ---------- emission schedule ----------
            # lead-in: first chunks in ko-halves so projections start early
            kv_load(0, 0)()
            q_load(0, 0)()
            kv_load(0, 1)()
            k_half(0, 0, 0)()
            k_half(0, 0, 1)()
            q_half(0, 0, 0)()
            q_half(0, 0, 1)()
            v_half(0, 0, 0)()
            v_half(0, 0, 1)()
            v_group(0, 0, 0)()
            v_group(0, 0, 1)()

            pre0 = {
                0: [kv_load(0, 2)],
                1: [k_half(0, 1, 0), k_half(0, 1, 1), kv_load(0, 3)],
                2: [v_half(0, 1, 0), v_half(0, 1, 1), q_load(0, 1)],
                3: [v_group(0, 1, 0), v_group(0, 1, 1)],
                5: [k_half(0, 2, 0), k_half(0, 2, 1)],
                6: [v_half(0, 2, 0), v_half(0, 2, 1)],
                7: [v_group(0, 2, 0), v_group(0, 2, 1)],
                9: [k_half(0, 3, 0), k_half(0, 3, 1)],
                10: [v_half(0, 3, 0), v_half(0, 3, 1), q_load(0, 2)],
                11: [v_group(0, 3, 0), v_group(0, 3, 1)],
                15: [q_half(0, 1, 0), q_half(0, 1, 1)],
                16: [q_load(0, 3)],
                20: [q_half(0, 2, 0), q_half(0, 2, 1)],
                22: [kv_load(1, 0)],
                26: [q_half(0, 3, 0), q_half(0, 3, 1)],
                28: [kv_load(1, 1)],
                33: [k_half(1, 0, 0), k_half(1, 0, 1)],
                34: [kv_load(1, 2)],
                35: [v_half(1, 0, 0), v_half(1, 0, 1)],
                36: [v_group(1, 0, 0), v_group(1, 0, 1)],
                39: [k_half(1, 1, 0), k_half(1, 1, 1)],
                40: [kv_load(1, 3)],
                41: [v_half(1, 1, 0), v_half(1, 1, 1)],
                42: [v_group(1, 1, 0), v_group(1, 1, 1)],
                45: [k_half(1, 2, 0), k_half(1, 2, 1)],
                46: [q_load(1, 0)],
                47: [v_half(1, 2, 0), v_half(1, 2, 1)],
                48: [v_group(1, 2, 0), v_group(1, 2, 1)],
                50: [q_load(1, 1)],
                51: [k_half(1, 3, 0), k_half(1, 3, 1)],
                53: [v_half(1, 3, 0), v_half(1, 3, 1)],
                54: [v_group(1, 3, 0), v_group(1, 3, 1), q_load(1, 2)],
                57: [q_half(1, 0, 0), q_half(1, 0, 1)],
                58: [q_load(1, 3)],
                59: [q_half(1, 1, 0), q_half(1, 1, 1)],
                61: [q_half(1, 2, 0), q_half(1, 2, 1)],
                63: [q_half(1, 3, 0), q_half(1, 3, 1)],
            }
            OPS = (8, 10, 12, 14)

            def add(d, key, fn):
                d.setdefault(key, []).append(fn)

            post0 = {}
            for qt in range(2):  # qt0 -> hooks at qt+2, qt1 -> qt+3
                add(post0, (qt + 2, 0), fb(0, qt))
                add(post0, (qt + 2, 6), fcd(0, qt))
                for i in range(4):
                    add(post0, (qt + 2, OPS[i]), outproj(0, qt * 4 + i))
            attention(0, pre0, post0)

            post1 = {}
            for qt in (2, 3):  # b0 qt2/qt3 spill into attention(1)
                add(post1, (qt - 2, 0), fb(0, qt))
                add(post1, (qt - 2, 6), fcd(0, qt))
                for i in range(4):
                    add(post1, (qt - 2, OPS[i]), outproj(0, qt * 4 + i))
            for qt in range(2):  # b1 qt0/qt1
                add(post1, (qt + 2, 0), fb(1, qt))
                add(post1, (qt + 2, 6), fcd(1, qt))
                for i in range(4):
                    add(post1, (qt + 2, OPS[i]), outproj(1, qt * 4 + i))
            add(post1, (3, 9), fb(1, 2))
            add(post1, (3, 13), fcd(1, 2))
            add(post1, (3, 14), outproj(1, 8))
            add(post1, (3, 15), outproj(1, 9))
            attention(1, {}, post1)
            # tail: remaining b1 tiles + the qt3 flush chain
            outproj(1, 10)()
            outproj(1, 11)()
            fb(1, 3)()
            fcd(1, 3)()
            for i in range(4):
                outproj(1, 12 + i, on_scalar=True)()

    nc.compile()
    return nc


def make_in_maps(x_q, x_kv, W_qkv, W_out):
    x_q = np.asarray(x_q, dtype=np.float32)
    x_kv = np.asarray(x_kv, dtype=np.float32)
    W_qkv = np.asarray(W_qkv, dtype=np.float32)
    W_out = np.asarray(W_out, dtype=np.float32)

    def chunk_tile(x):
        # [TOK, DIM] -> [n_chunks, 128, KO, PCHUNK] with D = ko*128 + p
        xt = x.reshape(TOK, DIM).T.reshape(KO, 128, TOK // PCHUNK, PCHUNK)
        return np.ascontiguousarray(xt.transpose(2, 1, 0, 3)).astype(BF)

    def w_tile(w):
        # [1024, HD] -> [128, KO, HD] with row = ko*128 + p
        return np.ascontiguousarray(
            w.reshape(KO, 128, HD).transpose(1, 0, 2)
        ).astype(BF)

    xqt = chunk_tile(x_q)
    xkvt = chunk_tile(x_kv)

    in_maps = []
    for c in range(N_CORES):
        cs = slice(c * HD, (c + 1) * HD)
        in_maps.append(
            {
                "xqt": xqt,
                "xkvt": xkvt,
                "wq": w_tile(W_qkv[:, cs]),
                "wk": w_tile(W_qkv[:, 1024:][:, cs]),
                "wv": w_tile(W_qkv[:, 2048:][:, cs]),
                "wout": np.ascontiguousarray(W_out[cs, :]).astype(BF),
            }
        )
    return in_maps


def combine(partials, b_out):
    """Sum the 8 per-core partial projections and add the bias."""
    acc = np.zeros((TOK, DIM), dtype=np.float32)
    for p in partials:
        acc += np.asarray(p, dtype=np.float32)
    acc += np.asarray(b_out, dtype=np.float32)
    return acc.reshape(B, SQ, DIM)


_STATE = {}


def _get_nc():
    if "nc" not in _STATE:
        _STATE["nc"] = build()
    return _STATE["nc"]


def run(x_q, x_kv, W_qkv, W_out, b_out, trace=False):
    nc = _get_nc()
    in_maps = make_in_maps(x_q, x_kv, W_qkv, W_out)
    res = run_bass_kernel_spmd(nc, in_maps, list(range(N_CORES)), trace=trace)
    out = combine([r["out"] for r in res.results], b_out)
    return out, res


def kernel(x_q, x_kv, W_qkv, W_out, b_out):
    out, _ = run(x_q, x_kv, W_qkv, W_out, b_out, trace=False)
    return out


# revision 22
# speedup vs baseline: 1.0010x; 1.0010x over previous
"""Multi-head attention (b=2, sq=skv=2048, dim=1024, 16 heads x 64) on 8 TRN2
NeuronCores.

Sharding: 2 heads per core (head-parallel across batch*heads), with the
matching tensor-parallel column slice of W_qkv and row slice of W_out.  Each
core computes a partial output projection over its 128 head-dims; the
all-reduce of the 8 partials (+ bias) happens on the host during unshard.

v3 design (ACT-bound steady state ~1us/step, PE/DVE trimmed to match):
  - scores: per (qt, j) one 2-bank PSUM tile [128 k, 2 h, 512 q]; the two
    heads' score matmuls (K=64) run concurrently in PE row-halves.
  - exp: one ACTIVATE per step over both heads (N=1024, scale 1/8 fused).
  - PV: col-tiled concurrent pair -- h0 -> acc[0:64], h1 -> acc[64:128]
    (tile_position (0,0)/(0,64) auto-derived), one PSUM bank per q-tile.
  - denominator: running sums of the exp tiles (13 adds/qt on DVE in bf16
    2x mode + 3 on the otherwise-idle Pool engine); at q-tile end two M=1
    ones-matmuls reduce them over k into two PSUM rows, which bounce
    through DRAM to repack [2,512] -> [128,8] so the DVE reciprocal runs
    wide (the 1-lane reciprocal costs 6.5ns/elem -- 3.3us/row), bounce
    back as bf16, and two concurrent K=1 outer-product matmuls broadcast
    r to the 64 partitions of each head.
  - normalization is fused into the acc PSUM->SBUF copy (tensor_mul).
  - v goes through the transposed projection (weight-stationary, N=512
    matmuls) + PE transposes into natural [k-token, dim] layout; the
    x-stationary variant costs ~220ns per N=128 matmul (LDWEIGHTS-bound)
    and is 2x more PE time.
  - emission: only chunk-0 projections precede attention(0); every other
    projection piece and outproj tile is drip-fed through per-step hooks
    (pre-hooks gate scores/PV inputs LOOKAHEAD steps early).  First
    chunks are loaded in ko-halves so projections start earlier; weights
    ride the scalar queue; kv chunks + flush bounces ride sync; q chunks
    + outputs ride gpsimd.
"""

import os
import sys

for _p in ("/opt/trn_rl_repo", "/root/.axon_site/_ro/trn_rl_repo"):
    if os.path.isdir(_p) and _p not in sys.path:
        sys.path.append(_p)

import ml_dtypes
import numpy as np

import concourse.bass as bass  # noqa: F401
import concourse.tile as tile
from concourse import bacc, mybir
from concourse.bass_utils import run_bass_kernel_spmd
from concourse.masks import make_identity

B, SQ, SKV, DIM = 2, 2048, 2048, 1024
HEADS, DH = 16, 64
N_CORES = 8
HPC = HEADS // N_CORES  # heads per core = 2
HD = HPC * DH  # 128 head-dim rows per core
TOK = B * SQ  # 4096
KO = DIM // 128  # 8 contraction chunks of 128
SCALE = DH**-0.5

BF16 = mybir.dt.bfloat16
F32 = mybir.dt.float32

PCHUNK = 512  # token chunk in projections (contiguous per-chunk dram layout)
QTILE = 512  # q tile in attention
KTILE = 128  # k tile (scores psum partition dim)
NKT = SKV // KTILE  # 16
NQT = SQ // QTILE  # 4
NCPB = SQ // PCHUNK  # chunks per batch = 4

BF = ml_dtypes.bfloat16
Exp = mybir.ActivationFunctionType.Exp

LOOKAHEAD = 6


def build():
    nc = bacc.Bacc(
        "TRN2", target_bir_lowering=False, debug=False, num_devices=N_CORES
    )

    NCH = TOK // PCHUNK
    xqt_d = nc.dram_tensor("xqt", [NCH, 128, KO, PCHUNK], BF16, kind="ExternalInput")
    xkvt_d = nc.dram_tensor("xkvt", [NCH, 128, KO, PCHUNK], BF16, kind="ExternalInput")
    wq_d = nc.dram_tensor("wq", [128, KO, HD], BF16, kind="ExternalInput")
    wk_d = nc.dram_tensor("wk", [128, KO, HD], BF16, kind="ExternalInput")
    wv_d = nc.dram_tensor("wv", [128, KO, HD], BF16, kind="ExternalInput")
    wout_d = nc.dram_tensor("wout", [HD, DIM], BF16, kind="ExternalInput")
    out_d = nc.dram_tensor("out", [TOK, DIM], BF16, kind="ExternalOutput")

    xqt = xqt_d.ap()
    xkvt = xkvt_d.ap()

    with tile.TileContext(nc) as tc:
        with (
            tc.tile_pool(name="persist", bufs=1) as persist,
            tc.tile_pool(name="xin", bufs=6) as xin,
            tc.tile_pool(name="exps", bufs=8) as exps,
            tc.tile_pool(name="spool", bufs=2) as spool,
            tc.tile_pool(name="usbp", bufs=3) as usbp,
            tc.tile_pool(name="rbp", bufs=2) as rbp,
            tc.tile_pool(name="bcp", bufs=2) as bcp,
            tc.tile_pool(name="dpkp", bufs=4) as dpkp,
            tc.tile_pool(name="dsp", bufs=2) as dsp,
            tc.tile_pool(name="ost", bufs=5) as ost,
            tc.tile_pool(name="spsum", bufs=2, space="PSUM") as spsum,
            tc.tile_pool(name="accp", bufs=2, space="PSUM") as accp,
            tc.tile_pool(name="miscp", bufs=2, space="PSUM") as miscp,
            tc.tile_pool(name="drp", bufs=2, space="DRAM") as drp,
        ):
            # --- weights on the scalar queue (its own DMA ring) ---
            wk_sb = persist.tile([128, KO, HD], BF16, tag="wk")
            nc.scalar.dma_start(wk_sb[:], wk_d.ap())
            wq_sb = persist.tile([128, KO, HD], BF16, tag="wq")
            nc.scalar.dma_start(wq_sb[:], wq_d.ap())
            wv_sb = persist.tile([128, KO, HD], BF16, tag="wv")
            nc.scalar.dma_start(wv_sb[:], wv_d.ap())
            wout_sb = persist.tile([HD, DIM], BF16, tag="wout")
            nc.scalar.dma_start(wout_sb[:], wout_d.ap())

            ones_col = persist.tile([128, 1], BF16, tag="ones_col")
            nc.vector.memset(ones_col[:], 1.0)
            ones_row = persist.tile([1, DH], BF16, tag="ones_row")
            nc.vector.memset(ones_row[:], 1.0)
            ident = persist.tile([128, DH], BF16, tag="ident")
            make_identity(nc, ident[0:DH, :])
            make_identity(nc, ident[DH : 2 * DH, :])
            # prefetch the exp table set during the initial DMAs
            dummy = persist.tile([1, 8], F32, tag="dummy")
            nc.vector.memset(dummy[:], 0.0)
            nc.scalar.activation(dummy[:], dummy[:], Exp)

            qt_sb, kt_sb, vt_sb, vnat, outT = {}, {}, {}, {}, {}
            for b in range(B):
                qt_sb[b] = persist.tile([HD, SQ], BF16, tag=f"qt{b}", name=f"qt{b}")
                kt_sb[b] = persist.tile([HD, SKV], BF16, tag=f"kt{b}", name=f"kt{b}")
                vt_sb[b] = persist.tile([HD, SKV], BF16, tag=f"vt{b}", name=f"vt{b}")
                vnat[b] = persist.tile(
                    [128, NKT, HD], BF16, tag=f"vn{b}", name=f"vn{b}"
                )
                outT[b] = persist.tile([HD, SQ], BF16, tag=f"ot{b}", name=f"ot{b}")

            # ---------- projection pieces ----------
            kv_tiles = {0: {}, 1: {}}
            q_tiles = {0: {}, 1: {}}
            projps = {}

            def _load(dst, x_ap, b, tt):
                # halves ride both DMA rings so no single transfer
                # exceeds ~5us and the rings stay load-balanced
                def go():
                    xt = xin.tile([128, KO, PCHUNK], BF16, tag="x")
                    src = x_ap[b * NCPB + tt]
                    nc.sync.dma_start(xt[:, 0:4, :], src[:, 0:4, :])
                    nc.gpsimd.dma_start(xt[:, 4:8, :], src[:, 4:8, :])
                    dst[b][tt] = xt

                return go

            def kv_load(b, tt):
                return _load(kv_tiles, xkvt, b, tt)

            def q_load(b, tt):
                return _load(q_tiles, xqt, b, tt)

            def proj_half(dst_d, w_sb, src_d, b, tt, half):
                """4 of the 8 ko-accumulation matmuls; copy on second half."""

                def go():
                    if half == 0:
                        projps[0] = miscp.tile(
                            [128, PCHUNK], F32, tag="m", name="projp"
                        )
                    ps = projps[0]
                    xt = src_d[b][tt]
                    for ko in range(half * 4, half * 4 + 4):
                        nc.tensor.matmul(
                            ps[:],
                            w_sb[:, ko, :],
                            xt[:, ko, :],
                            start=(ko == 0),
                            stop=(ko == KO - 1),
                        )
                    if half == 1:
                        t0 = tt * PCHUNK
                        nc.vector.tensor_copy(
                            dst_d[b][:, t0 : t0 + PCHUNK], ps[:]
                        )

                return go

            def k_half(b, tt, half):
                return proj_half(kt_sb, wk_sb, kv_tiles, b, tt, half)

            def q_half(b, tt, half):
                return proj_half(qt_sb, wq_sb, q_tiles, b, tt, half)

            def v_half(b, tt, half):
                return proj_half(vt_sb, wv_sb, kv_tiles, b, tt, half)

            def v_group(b, jg, h):
                """PE-transpose k-tiles 4jg..4jg+3 of head h of vT into
                natural [k-token, dim] layout."""

                def go():
                    h_sl = slice(h * DH, (h + 1) * DH)
                    tp = miscp.tile([128, 4, DH], BF16, tag="m", name="vtp")
                    for i in range(4):
                        j = jg * 4 + i
                        nc.tensor.transpose(
                            tp[:, i, :],
                            vt_sb[b][h_sl, j * KTILE : (j + 1) * KTILE],
                            ident[h_sl, :],
                        )
                    nc.vector.tensor_copy(
                        vnat[b][:, jg * 4 : (jg + 1) * 4, h_sl], tp[:]
                    )

                return go

            # ---------- flush (denominator + normalization) ----------
            # Four stages scheduled ~steps apart via hooks so the in-order
            # PE/DVE queues never wait on the DRAM repack round-trip.
            def flush_a(b, qt, acc, S):
                """j==15: reduce S over k, stage everything out of PSUM,
                kick off the repack DMAs.  Frees acc + dsum immediately."""
                dsum = miscp.tile([128, QTILE], F32, tag="m", name="dsum")
                for h in range(HPC):
                    nc.tensor.matmul(
                        dsum[h * 32 : h * 32 + 1, :],
                        ones_col[:],
                        S[:, h, :],
                        start=True,
                        stop=True,
                        skip_group_check=True,
                    )
                dstage = dsp.tile([33, QTILE], F32, tag="ds", name="dstage")
                for h in range(HPC):
                    nc.vector.tensor_copy(
                        dstage[h * 32 : h * 32 + 1, :],
                        dsum[h * 32 : h * 32 + 1, :],
                    )
                # unnormalized accumulator out of PSUM (bf16; the later
                # normalizing multiply runs SBUF-only in 2x mode)
                usb = usbp.tile([128, QTILE], BF16, tag="u", name="usb")
                nc.vector.tensor_copy(usb[:], acc[:])
                d1 = drp.tile([HPC, QTILE], F32, tag="d1", name="d1")
                for h in range(HPC):
                    nc.sync.dma_start(
                        d1[h : h + 1, :], dstage[h * 32 : h * 32 + 1, :]
                    )
                dpk = dpkp.tile([128, HPC, 4], F32, tag="dp", name="dpk")
                for h in range(HPC):
                    nc.sync.dma_start(
                        dpk[:, h, :],
                        d1[h, :].rearrange("(p f) -> p f", p=128),
                    )
                return usb, dpk

            pending = {}

            def fb(b, qt):
                """~2 q-tiles later: dpk has landed; wide reciprocal, send
                the bf16 reciprocals back to a [1, 2, 512] row."""

                def go():
                    st = pending[(b, qt)]
                    dpk = st[3]
                    rpk = dpkp.tile([128, HPC, 4], BF16, tag="rp", name="rpk")
                    with nc.allow_low_precision(reason="bf16 softmax recip"):
                        nc.vector.reciprocal(rpk[:], dpk[:])
                    d2 = drp.tile([HPC, QTILE], BF16, tag="d2", name="d2")
                    for h in range(HPC):
                        nc.sync.dma_start(
                            d2[h, :].rearrange("(p f) -> p f", p=128),
                            rpk[:, h, :],
                        )
                    rb = rbp.tile([1, HPC, QTILE], BF16, tag="rb", name="rb")
                    nc.sync.dma_start(rb[0:1, :, :], d2[:])
                    st.append(rb)

                return go

            def fcd(b, qt):
                """broadcast r to the 64 partitions of each head (K=1
                outer products, concurrent col halves), then the
                normalizing multiply into outT (all-SBUF, 2x mode)."""

                def go():
                    st = pending.pop((b, qt))
                    _b, _qt, usb, _dpk, rb = st
                    q_sl = slice(qt * QTILE, (qt + 1) * QTILE)
                    bc = miscp.tile([128, QTILE], F32, tag="m", name="bc")
                    for h in range(HPC):
                        nc.tensor.matmul(
                            bc[h * DH : (h + 1) * DH, :],
                            ones_row[:],
                            rb[0:1, h, :],
                            start=True,
                            stop=True,
                            skip_group_check=True,
                        )
                    bcs = bcp.tile([128, QTILE], BF16, tag="bc", name="bcs")
                    nc.vector.tensor_copy(bcs[:], bc[:])
                    for h in range(HPC):
                        h_sl = slice(h * DH, (h + 1) * DH)
                        nc.vector.tensor_mul(
                            outT[b][h_sl, q_sl], usb[h_sl, :], bcs[h_sl, :]
                        )

                return go

            # ---------- output projection ----------
            def outproj(b, tt, on_scalar=False):
                def go():
                    t_sl = slice(tt * 128, (tt + 1) * 128)
                    ob = ost.tile([128, 2, 512], BF16, tag="o")
                    for nt in range(DIM // 512):
                        ps = miscp.tile([128, PCHUNK], F32, tag="m", name="projo")
                        nc.tensor.matmul(
                            ps[:],
                            outT[b][:, t_sl],
                            wout_sb[:, nt * 512 : (nt + 1) * 512],
                            start=True,
                            stop=True,
                        )
                        if on_scalar:
                            nc.scalar.copy(ob[:, nt, :], ps[:])
                        else:
                            nc.vector.tensor_copy(ob[:, nt, :], ps[:])
                    dq = nc.scalar if on_scalar else nc.gpsimd
                    dq.dma_start(
                        out_d.ap()[
                            b * SQ + tt * 128 : b * SQ + (tt + 1) * 128, :
                        ].rearrange("t (n c) -> t n c", n=2),
                        ob[:],
                    )

                return go

            # ---------- attention ----------
            def attention(b, pre_hooks, post_hooks):
                NT = NQT * NKT
                sps, st, accs = {}, {}, {}

                def emit_scores(t):
                    qt, j = divmod(t, NKT)
                    q_sl = slice(qt * QTILE, (qt + 1) * QTILE)
                    k_sl = slice(j * KTILE, (j + 1) * KTILE)
                    sp = spsum.tile([128, HPC, QTILE], F32, tag="s", name="sp")
                    sps[t] = sp
                    for h in range(HPC):
                        h_sl = slice(h * DH, (h + 1) * DH)
                        nc.tensor.matmul(
                            sp[:, h, :],
                            kt_sb[b][h_sl, k_sl],
                            qt_sb[b][h_sl, q_sl],
                            start=True,
                            stop=True,
                        )

                def emit_tail(t):
                    qt, j = divmod(t, NKT)
                    sp = sps.pop(t)
                    ex = exps.tile([128, HPC, QTILE], BF16, tag="e", name="ex")
                    nc.scalar.activation(ex[:], sp[:], Exp, scale=SCALE)
                    if j == 0:
                        accs[qt] = accp.tile(
                            [128, QTILE], F32, tag="acc", name="acc"
                        )
                        st[qt] = spool.tile(
                            [128, HPC, QTILE], BF16, tag="S", name="S"
                        )
                        nc.vector.tensor_copy(st[qt][:], ex[:])
                    else:
                        nc.vector.tensor_add(st[qt][:], st[qt][:], ex[:])
                    for h in range(HPC):
                        nc.tensor.matmul(
                            accs[qt][h * DH : (h + 1) * DH, :],
                            vnat[b][:, j, h * DH : (h + 1) * DH],
                            ex[:, h, :],
                            start=(j == 0),
                            stop=(j == NKT - 1),
                            skip_group_check=True,
                        )
                    if j == NKT - 1:
                        usb, dpk = flush_a(b, qt, accs.pop(qt), st.pop(qt))
                        pending[(b, qt)] = [b, qt, usb, dpk]
                    for fn in post_hooks.get((qt, j), []):
                        fn()

                for t in range(NT + LOOKAHEAD):
                    if t < NT:
                        for fn in pre_hooks.get(t, []):
                            fn()
                        emit_scores(t)
                    if t >= LOOKAHEAD:
                        emit_tail(t - LOOKAHEAD)

            # ---------- emission schedule ----------
            # lead-in: first chunks in ko-halves so projections start early
            kv_load(0, 0)()
            q_load(0, 0)()
            kv_load(0, 1)()
            k_half(0, 0, 0)()
            k_half(0, 0, 1)()
            q_half(0, 0, 0)()
            q_half(0, 0, 1)()
            v_half(0, 0, 0)()
            v_half(0, 0, 1)()
            v_group(0, 0, 0)()
            v_group(0, 0, 1)()

            pre0 = {
                0: [kv_load(0, 2)],
                1: [k_half(0, 1, 0), k_half(0, 1, 1), kv_load(0, 3)],
                2: [v_half(0, 1, 0), v_half(0, 1, 1), q_load(0, 1)],
                3: [v_group(0, 1, 0), v_group(0, 1, 1)],
                5: [k_half(0, 2, 0), k_half(0, 2, 1)],
                6: [v_half(0, 2, 0), v_half(0, 2, 1)],
                7: [v_group(0, 2, 0), v_group(0, 2, 1)],
                9: [k_half(0, 3, 0), k_half(0, 3, 1)],
                10: [v_half(0, 3, 0), v_half(0, 3, 1), q_load(0, 2)],
                11: [v_group(0, 3, 0), v_group(0, 3, 1)],
                15: [q_half(0, 1, 0), q_half(0, 1, 1)],
                16: [q_load(0, 3)],
                20: [q_half(0, 2, 0), q_half(0, 2, 1)],
                22: [kv_load(1, 0)],
                26: [q_half(0, 3, 0), q_half(0, 3, 1)],
                28: [kv_load(1, 1)],
                33: [k_half(1, 0, 0), k_half(1, 0, 1)],
                34: [kv_load(1, 2)],
                35: [v_half(1, 0, 0), v_half(1, 0, 1)],
                36: [v_group(1, 0, 0), v_group(1, 0, 1)],
                39: [k_half(1, 1, 0), k_half(1, 1, 1)],
                40: [kv_load(1, 3)],
                41: [v_half(1, 1, 0), v_half(1, 1, 1)],
                42: [v_group(1, 1, 0), v_group(1, 1, 1)],
                45: [k_half(1, 2, 0), k_half(1, 2, 1)],
                46: [q_load(1, 0)],
                47: [v_half(1, 2, 0), v_half(1, 2, 1)],
                48: [v_group(1, 2, 0), v_group(1, 2, 1)],
                50: [q_load(1, 1)],
                54: [q_load(1, 2)],
                57: [q_half(1, 0, 0), q_half(1, 0, 1)],
                58: [q_load(1, 3)],
            }
            OPS = (8, 10, 12, 14)

            def add(d, key, fn):
                d.setdefault(key, []).append(fn)

            post0 = {}
            for qt in range(2):  # qt0 -> hooks at qt+2, qt1 -> qt+3
                add(post0, (qt + 2, 0), fb(0, qt))
                add(post0, (qt + 2, 6), fcd(0, qt))
                for i in range(4):
                    add(post0, (qt + 2, OPS[i]), outproj(0, qt * 4 + i))
            attention(0, pre0, post0)

            pre1 = {
                2: [q_half(1, 1, 0), q_half(1, 1, 1)],
                4: [k_half(1, 3, 0), k_half(1, 3, 1)],
                6: [v_half(1, 3, 0), v_half(1, 3, 1)],
                8: [v_group(1, 3, 0), v_group(1, 3, 1)],
                18: [q_half(1, 2, 0), q_half(1, 2, 1)],
                34: [q_half(1, 3, 0), q_half(1, 3, 1)],
            }
            post1 = {}
            for qt in (2, 3):  # b0 qt2/qt3 spill into attention(1)
                add(post1, (qt - 2, 0), fb(0, qt))
                add(post1, (qt - 2, 6), fcd(0, qt))
                for i in range(4):
                    add(post1, (qt - 2, OPS[i]), outproj(0, qt * 4 + i))
            for qt in range(2):  # b1 qt0/qt1
                add(post1, (qt + 2, 0), fb(1, qt))
                add(post1, (qt + 2, 6), fcd(1, qt))
                for i in range(4):
                    add(post1, (qt + 2, OPS[i]), outproj(1, qt * 4 + i))
            add(post1, (3, 4), fb(1, 2))
            add(post1, (3, 8), fcd(1, 2))
            add(post1, (3, 9), outproj(1, 8))
            add(post1, (3, 10), outproj(1, 9))
            add(post1, (3, 12), outproj(1, 10))
            add(post1, (3, 14), outproj(1, 11))
            attention(1, pre1, post1)
            # tail: the qt3 flush chain + last tiles (scalar ring)
            fb(1, 3)()
            fcd(1, 3)()
            for i in range(4):
                outproj(1, 12 + i, on_scalar=True)()

    nc.compile()
    return nc


def make_in_maps(x_q, x_kv, W_qkv, W_out):
    x_q = np.asarray(x_q, dtype=np.float32)
    x_kv = np.asarray(x_kv, dtype=np.float32)
    W_qkv = np.asarray(W_qkv, dtype=np.float32)
    W_out = np.asarray(W_out, dtype=np.float32)

    def chunk_tile(x):
        # [TOK, DIM] -> [n_chunks, 128, KO, PCHUNK] with D = ko*128 + p
        xt = x.reshape(TOK, DIM).T.reshape(KO, 128, TOK // PCHUNK, PCHUNK)
        return np.ascontiguousarray(xt.transpose(2, 1, 0, 3)).astype(BF)

    def w_tile(w):
        # [1024, HD] -> [128, KO, HD] with row = ko*128 + p
        return np.ascontiguousarray(
            w.reshape(KO, 128, HD).transpose(1, 0, 2)
        ).astype(BF)

    xqt = chunk_tile(x_q)
    xkvt = chunk_tile(x_kv)

    in_maps = []
    for c in range(N_CORES):
        cs = slice(c * HD, (c + 1) * HD)
        in_maps.append(
            {
                "xqt": xqt,
                "xkvt": xkvt,
                "wq": w_tile(W_qkv[:, cs]),
                "wk": w_tile(W_qkv[:, 1024:][:, cs]),
                "wv": w_tile(W_qkv[:, 2048:][:, cs]),
                "wout": np.ascontiguousarray(W_out[cs, :]).astype(BF),
            }
        )
    return in_maps


def combine(partials, b_out):
    """Sum the 8 per-core partial projections and add the bias."""
    acc = np.zeros((TOK, DIM), dtype=np.float32)
    for p in partials:
        acc += np.asarray(p, dtype=np.float32)
    acc += np.asarray(b_out, dtype=np.float32)
    return acc.reshape(B, SQ, DIM)


_STATE = {}


def _get_nc():
    if "nc" not in _STATE:
        _STATE["nc"] = build()
    return _STATE["nc"]


def run(x_q, x_kv, W_qkv, W_out, b_out, trace=False):
    nc = _get_nc()
    in_maps = make_in_maps(x_q, x_kv, W_qkv, W_out)
    res = run_bass_kernel_spmd(nc, in_maps, list(range(N_CORES)), trace=trace)
    out = combine([r["out"] for r in res.results], b_out)
    return out, res


def kernel(x_q, x_kv, W_qkv, W_out, b_out):
    out, _ = run(x_q, x_kv, W_qkv, W_out, b_out, trace=False)
    return out


# revision 23
# speedup vs baseline: 1.0267x; 1.0257x over previous
"""Multi-head attention (b=2, sq=skv=2048, dim=1024, 16 heads x 64) on 8 TRN2
NeuronCores.

Sharding: 2 heads per core (head-parallel across batch*heads), with the
matching tensor-parallel column slice of W_qkv and row slice of W_out.  Each
core computes a partial output projection over its 128 head-dims; the
all-reduce of the 8 partials (+ bias) happens on the host during unshard.

v3 design (ACT-bound steady state ~1us/step, PE/DVE trimmed to match):
  - scores: per (qt, j) one 2-bank PSUM tile [128 k, 2 h, 512 q]; the two
    heads' score matmuls (K=64) run concurrently in PE row-halves.
  - exp: one ACTIVATE per step over both heads (N=1024, scale 1/8 fused).
  - PV: col-tiled concurrent pair -- h0 -> acc[0:64], h1 -> acc[64:128]
    (tile_position (0,0)/(0,64) auto-derived), one PSUM bank per q-tile.
  - denominator: running sums of the exp tiles (13 adds/qt on DVE in bf16
    2x mode + 3 on the otherwise-idle Pool engine); at q-tile end two M=1
    ones-matmuls reduce them over k into two PSUM rows, which bounce
    through DRAM to repack [2,512] -> [128,8] so the DVE reciprocal runs
    wide (the 1-lane reciprocal costs 6.5ns/elem -- 3.3us/row), bounce
    back as bf16, and two concurrent K=1 outer-product matmuls broadcast
    r to the 64 partitions of each head.
  - normalization is fused into the acc PSUM->SBUF copy (tensor_mul).
  - v goes through the transposed projection (weight-stationary, N=512
    matmuls) + PE transposes into natural [k-token, dim] layout; the
    x-stationary variant costs ~220ns per N=128 matmul (LDWEIGHTS-bound)
    and is 2x more PE time.
  - emission: only chunk-0 projections precede attention(0); every other
    projection piece and outproj tile is drip-fed through per-step hooks
    (pre-hooks gate scores/PV inputs LOOKAHEAD steps early).  First
    chunks are loaded in ko-halves so projections start earlier; weights
    ride the scalar queue; kv chunks + flush bounces ride sync; q chunks
    + outputs ride gpsimd.
"""

import os
import sys

for _p in ("/opt/trn_rl_repo", "/root/.axon_site/_ro/trn_rl_repo"):
    if os.path.isdir(_p) and _p not in sys.path:
        sys.path.append(_p)

import ml_dtypes
import numpy as np

import concourse.bass as bass  # noqa: F401
import concourse.tile as tile
from concourse import bacc, mybir
from concourse.bass_utils import run_bass_kernel_spmd
from concourse.masks import make_identity

B, SQ, SKV, DIM = 2, 2048, 2048, 1024
HEADS, DH = 16, 64
N_CORES = 8
HPC = HEADS // N_CORES  # heads per core = 2
HD = HPC * DH  # 128 head-dim rows per core
TOK = B * SQ  # 4096
KO = DIM // 128  # 8 contraction chunks of 128
SCALE = DH**-0.5

BF16 = mybir.dt.bfloat16
F32 = mybir.dt.float32

PCHUNK = 512  # token chunk in projections (contiguous per-chunk dram layout)
QTILE = 512  # q tile in attention
KTILE = 128  # k tile (scores psum partition dim)
NKT = SKV // KTILE  # 16
NQT = SQ // QTILE  # 4
NCPB = SQ // PCHUNK  # chunks per batch = 4

BF = ml_dtypes.bfloat16
Exp = mybir.ActivationFunctionType.Exp

LOOKAHEAD = 6


def build():
    nc = bacc.Bacc(
        "TRN2", target_bir_lowering=False, debug=False, num_devices=N_CORES
    )

    NCH = TOK // PCHUNK
    xqt_d = nc.dram_tensor("xqt", [NCH, 128, KO, PCHUNK], BF16, kind="ExternalInput")
    xkvt_d = nc.dram_tensor("xkvt", [NCH, 128, KO, PCHUNK], BF16, kind="ExternalInput")
    wq_d = nc.dram_tensor("wq", [128, KO, HD], BF16, kind="ExternalInput")
    wk_d = nc.dram_tensor("wk", [128, KO, HD], BF16, kind="ExternalInput")
    wv_d = nc.dram_tensor("wv", [128, KO, HD], BF16, kind="ExternalInput")
    wout_d = nc.dram_tensor("wout", [HD, DIM], BF16, kind="ExternalInput")
    out_d = nc.dram_tensor("out", [TOK, DIM], BF16, kind="ExternalOutput")

    xqt = xqt_d.ap()
    xkvt = xkvt_d.ap()

    with tile.TileContext(nc) as tc:
        with (
            tc.tile_pool(name="persist", bufs=1) as persist,
            tc.tile_pool(name="xin", bufs=6) as xin,
            tc.tile_pool(name="exps", bufs=8) as exps,
            tc.tile_pool(name="spool", bufs=2) as spool,
            tc.tile_pool(name="usbp", bufs=3) as usbp,
            tc.tile_pool(name="rbp", bufs=2) as rbp,
            tc.tile_pool(name="bcp", bufs=2) as bcp,
            tc.tile_pool(name="dpkp", bufs=4) as dpkp,
            tc.tile_pool(name="dsp", bufs=2) as dsp,
            tc.tile_pool(name="ost", bufs=5) as ost,
            tc.tile_pool(name="spsum", bufs=2, space="PSUM") as spsum,
            tc.tile_pool(name="accp", bufs=2, space="PSUM") as accp,
            tc.tile_pool(name="miscp", bufs=2, space="PSUM") as miscp,
            tc.tile_pool(name="drp", bufs=2, space="DRAM") as drp,
        ):
            # --- weights on the scalar queue (its own DMA ring) ---
            wk_sb = persist.tile([128, KO, HD], BF16, tag="wk")
            nc.scalar.dma_start(wk_sb[:], wk_d.ap())
            wq_sb = persist.tile([128, KO, HD], BF16, tag="wq")
            nc.scalar.dma_start(wq_sb[:], wq_d.ap())
            wv_sb = persist.tile([128, KO, HD], BF16, tag="wv")
            nc.scalar.dma_start(wv_sb[:], wv_d.ap())
            wout_sb = persist.tile([HD, DIM], BF16, tag="wout")
            nc.scalar.dma_start(wout_sb[:], wout_d.ap())

            ones_col = persist.tile([128, 1], BF16, tag="ones_col")
            nc.vector.memset(ones_col[:], 1.0)
            ones_row = persist.tile([1, DH], BF16, tag="ones_row")
            nc.vector.memset(ones_row[:], 1.0)
            ident = persist.tile([128, DH], BF16, tag="ident")
            make_identity(nc, ident[0:DH, :])
            make_identity(nc, ident[DH : 2 * DH, :])
            # prefetch the exp table set during the initial DMAs
            dummy = persist.tile([1, 8], F32, tag="dummy")
            nc.vector.memset(dummy[:], 0.0)
            nc.scalar.activation(dummy[:], dummy[:], Exp)

            qt_sb, kt_sb, vt_sb, vnat, outT = {}, {}, {}, {}, {}
            for b in range(B):
                qt_sb[b] = persist.tile([HD, SQ], BF16, tag=f"qt{b}", name=f"qt{b}")
                kt_sb[b] = persist.tile([HD, SKV], BF16, tag=f"kt{b}", name=f"kt{b}")
                vt_sb[b] = persist.tile([HD, SKV], BF16, tag=f"vt{b}", name=f"vt{b}")
                vnat[b] = persist.tile(
                    [128, NKT, HD], BF16, tag=f"vn{b}", name=f"vn{b}"
                )
                outT[b] = persist.tile([HD, SQ], BF16, tag=f"ot{b}", name=f"ot{b}")

            # ---------- projection pieces ----------
            kv_tiles = {0: {}, 1: {}}
            q_tiles = {0: {}, 1: {}}
            projps = {}

            def _load(dst, x_ap, b, tt, quarters=False):
                # halves ride both DMA rings so no single transfer
                # exceeds ~5us and the rings stay load-balanced; the very
                # first chunks go as quarters for the fastest arrival
                def go():
                    xt = xin.tile([128, KO, PCHUNK], BF16, tag="x")
                    src = x_ap[b * NCPB + tt]
                    if quarters:
                        nc.sync.dma_start(xt[:, 0:2, :], src[:, 0:2, :])
                        nc.gpsimd.dma_start(xt[:, 2:4, :], src[:, 2:4, :])
                        nc.sync.dma_start(xt[:, 4:6, :], src[:, 4:6, :])
                        nc.gpsimd.dma_start(xt[:, 6:8, :], src[:, 6:8, :])
                    else:
                        nc.sync.dma_start(xt[:, 0:4, :], src[:, 0:4, :])
                        nc.gpsimd.dma_start(xt[:, 4:8, :], src[:, 4:8, :])
                    dst[b][tt] = xt

                return go

            def kv_load(b, tt, quarters=False):
                return _load(kv_tiles, xkvt, b, tt, quarters)

            def q_load(b, tt, quarters=False):
                return _load(q_tiles, xqt, b, tt, quarters)

            def proj_half(dst_d, w_sb, src_d, b, tt, half):
                """4 of the 8 ko-accumulation matmuls; copy on second half."""

                def go():
                    if half == 0:
                        projps[0] = miscp.tile(
                            [128, PCHUNK], F32, tag="m", name="projp"
                        )
                    ps = projps[0]
                    xt = src_d[b][tt]
                    for ko in range(half * 4, half * 4 + 4):
                        nc.tensor.matmul(
                            ps[:],
                            w_sb[:, ko, :],
                            xt[:, ko, :],
                            start=(ko == 0),
                            stop=(ko == KO - 1),
                        )
                    if half == 1:
                        t0 = tt * PCHUNK
                        nc.vector.tensor_copy(
                            dst_d[b][:, t0 : t0 + PCHUNK], ps[:]
                        )

                return go

            def k_half(b, tt, half):
                return proj_half(kt_sb, wk_sb, kv_tiles, b, tt, half)

            def q_half(b, tt, half):
                return proj_half(qt_sb, wq_sb, q_tiles, b, tt, half)

            def v_half(b, tt, half):
                return proj_half(vt_sb, wv_sb, kv_tiles, b, tt, half)

            def v_group(b, jg, h):
                """PE-transpose k-tiles 4jg..4jg+3 of head h of vT into
                natural [k-token, dim] layout."""

                def go():
                    h_sl = slice(h * DH, (h + 1) * DH)
                    tp = miscp.tile([128, 4, DH], BF16, tag="m", name="vtp")
                    for i in range(4):
                        j = jg * 4 + i
                        nc.tensor.transpose(
                            tp[:, i, :],
                            vt_sb[b][h_sl, j * KTILE : (j + 1) * KTILE],
                            ident[h_sl, :],
                        )
                    nc.vector.tensor_copy(
                        vnat[b][:, jg * 4 : (jg + 1) * 4, h_sl], tp[:]
                    )

                return go

            # ---------- flush (denominator + normalization) ----------
            # Four stages scheduled ~steps apart via hooks so the in-order
            # PE/DVE queues never wait on the DRAM repack round-trip.
            def flush_a(b, qt, acc, S):
                """j==15: reduce S over k, stage everything out of PSUM,
                kick off the repack DMAs.  Frees acc + dsum immediately."""
                dsum = miscp.tile([128, QTILE], F32, tag="m", name="dsum")
                for h in range(HPC):
                    nc.tensor.matmul(
                        dsum[h * 32 : h * 32 + 1, :],
                        ones_col[:],
                        S[:, h, :],
                        start=True,
                        stop=True,
                        skip_group_check=True,
                    )
                dstage = dsp.tile([33, QTILE], F32, tag="ds", name="dstage")
                for h in range(HPC):
                    nc.vector.tensor_copy(
                        dstage[h * 32 : h * 32 + 1, :],
                        dsum[h * 32 : h * 32 + 1, :],
                    )
                # unnormalized accumulator out of PSUM (bf16; the later
                # normalizing multiply runs SBUF-only in 2x mode)
                usb = usbp.tile([128, QTILE], BF16, tag="u", name="usb")
                nc.vector.tensor_copy(usb[:], acc[:])
                d1 = drp.tile([HPC, QTILE], F32, tag="d1", name="d1")
                for h in range(HPC):
                    nc.sync.dma_start(
                        d1[h : h + 1, :], dstage[h * 32 : h * 32 + 1, :]
                    )
                dpk = dpkp.tile([128, HPC, 4], F32, tag="dp", name="dpk")
                for h in range(HPC):
                    nc.sync.dma_start(
                        dpk[:, h, :],
                        d1[h, :].rearrange("(p f) -> p f", p=128),
                    )
                return usb, dpk

            pending = {}

            def fb(b, qt):
                """~2 q-tiles later: dpk has landed; wide reciprocal, send
                the bf16 reciprocals back to a [1, 2, 512] row."""

                def go():
                    st = pending[(b, qt)]
                    dpk = st[3]
                    rpk = dpkp.tile([128, HPC, 4], BF16, tag="rp", name="rpk")
                    with nc.allow_low_precision(reason="bf16 softmax recip"):
                        nc.vector.reciprocal(rpk[:], dpk[:])
                    d2 = drp.tile([HPC, QTILE], BF16, tag="d2", name="d2")
                    for h in range(HPC):
                        nc.sync.dma_start(
                            d2[h, :].rearrange("(p f) -> p f", p=128),
                            rpk[:, h, :],
                        )
                    rb = rbp.tile([1, HPC, QTILE], BF16, tag="rb", name="rb")
                    nc.sync.dma_start(rb[0:1, :, :], d2[:])
                    st.append(rb)

                return go

            def fcd(b, qt):
                """broadcast r to the 64 partitions of each head (K=1
                outer products, concurrent col halves), then the
                normalizing multiply into outT (all-SBUF, 2x mode)."""

                def go():
                    st = pending.pop((b, qt))
                    _b, _qt, usb, _dpk, rb = st
                    q_sl = slice(qt * QTILE, (qt + 1) * QTILE)
                    bc = miscp.tile([128, QTILE], F32, tag="m", name="bc")
                    for h in range(HPC):
                        nc.tensor.matmul(
                            bc[h * DH : (h + 1) * DH, :],
                            ones_row[:],
                            rb[0:1, h, :],
                            start=True,
                            stop=True,
                            skip_group_check=True,
                        )
                    bcs = bcp.tile([128, QTILE], BF16, tag="bc", name="bcs")
                    nc.vector.tensor_copy(bcs[:], bc[:])
                    for h in range(HPC):
                        h_sl = slice(h * DH, (h + 1) * DH)
                        nc.vector.tensor_mul(
                            outT[b][h_sl, q_sl], usb[h_sl, :], bcs[h_sl, :]
                        )

                return go

            # ---------- output projection ----------
            def outproj(b, tt, mode="dve"):
                def go():
                    t_sl = slice(tt * 128, (tt + 1) * 128)
                    ob = ost.tile([128, 2, 512], BF16, tag="o")
                    for nt in range(DIM // 512):
                        ps = miscp.tile([128, PCHUNK], F32, tag="m", name="projo")
                        nc.tensor.matmul(
                            ps[:],
                            outT[b][:, t_sl],
                            wout_sb[:, nt * 512 : (nt + 1) * 512],
                            start=True,
                            stop=True,
                        )
                        if mode == "scalar" or (mode == "split" and nt == 0):
                            nc.scalar.copy(ob[:, nt, :], ps[:])
                        else:
                            nc.vector.tensor_copy(ob[:, nt, :], ps[:])
                    dq = nc.scalar if mode == "scalar" else nc.gpsimd
                    dq.dma_start(
                        out_d.ap()[
                            b * SQ + tt * 128 : b * SQ + (tt + 1) * 128, :
                        ].rearrange("t (n c) -> t n c", n=2),
                        ob[:],
                    )

                return go

            # ---------- attention ----------
            def attention(b, pre_hooks, post_hooks):
                NT = NQT * NKT
                sps, st, accs = {}, {}, {}

                def emit_scores(t):
                    qt, j = divmod(t, NKT)
                    q_sl = slice(qt * QTILE, (qt + 1) * QTILE)
                    k_sl = slice(j * KTILE, (j + 1) * KTILE)
                    sp = spsum.tile([128, HPC, QTILE], F32, tag="s", name="sp")
                    sps[t] = sp
                    for h in range(HPC):
                        h_sl = slice(h * DH, (h + 1) * DH)
                        nc.tensor.matmul(
                            sp[:, h, :],
                            kt_sb[b][h_sl, k_sl],
                            qt_sb[b][h_sl, q_sl],
                            start=True,
                            stop=True,
                        )

                def emit_tail(t):
                    qt, j = divmod(t, NKT)
                    sp = sps.pop(t)
                    ex = exps.tile([128, HPC, QTILE], BF16, tag="e", name="ex")
                    nc.scalar.activation(ex[:], sp[:], Exp, scale=SCALE)
                    if j == 0:
                        accs[qt] = accp.tile(
                            [128, QTILE], F32, tag="acc", name="acc"
                        )
                        st[qt] = spool.tile(
                            [128, HPC, QTILE], BF16, tag="S", name="S"
                        )
                        nc.vector.tensor_copy(st[qt][:], ex[:])
                    else:
                        nc.vector.tensor_add(st[qt][:], st[qt][:], ex[:])
                    for h in range(HPC):
                        nc.tensor.matmul(
                            accs[qt][h * DH : (h + 1) * DH, :],
                            vnat[b][:, j, h * DH : (h + 1) * DH],
                            ex[:, h, :],
                            start=(j == 0),
                            stop=(j == NKT - 1),
                            skip_group_check=True,
                        )
                    if j == NKT - 1:
                        usb, dpk = flush_a(b, qt, accs.pop(qt), st.pop(qt))
                        pending[(b, qt)] = [b, qt, usb, dpk]
                    for fn in post_hooks.get((qt, j), []):
                        fn()

                for t in range(NT + LOOKAHEAD):
                    if t < NT:
                        for fn in pre_hooks.get(t, []):
                            fn()
                        emit_scores(t)
                    if t >= LOOKAHEAD:
                        emit_tail(t - LOOKAHEAD)

            # ---------- emission schedule ----------
            # lead-in: first chunks in ko-halves so projections start early
            kv_load(0, 0, quarters=True)()
            q_load(0, 0, quarters=True)()
            kv_load(0, 1)()
            k_half(0, 0, 0)()
            k_half(0, 0, 1)()
            q_half(0, 0, 0)()
            q_half(0, 0, 1)()
            v_half(0, 0, 0)()
            v_half(0, 0, 1)()
            v_group(0, 0, 0)()
            v_group(0, 0, 1)()

            pre0 = {
                0: [kv_load(0, 2)],
                1: [k_half(0, 1, 0), k_half(0, 1, 1), kv_load(0, 3)],
                2: [v_half(0, 1, 0), v_half(0, 1, 1), q_load(0, 1)],
                3: [v_group(0, 1, 0), v_group(0, 1, 1)],
                5: [k_half(0, 2, 0), k_half(0, 2, 1)],
                6: [v_half(0, 2, 0), v_half(0, 2, 1)],
                7: [v_group(0, 2, 0), v_group(0, 2, 1)],
                9: [k_half(0, 3, 0), k_half(0, 3, 1)],
                10: [v_half(0, 3, 0), v_half(0, 3, 1), q_load(0, 2)],
                11: [v_group(0, 3, 0), v_group(0, 3, 1)],
                15: [q_half(0, 1, 0), q_half(0, 1, 1)],
                16: [q_load(0, 3)],
                20: [q_half(0, 2, 0), q_half(0, 2, 1)],
                22: [kv_load(1, 0)],
                26: [q_half(0, 3, 0), q_half(0, 3, 1)],
                28: [kv_load(1, 1)],
                33: [k_half(1, 0, 0), k_half(1, 0, 1)],
                34: [kv_load(1, 2)],
                35: [v_half(1, 0, 0), v_half(1, 0, 1)],
                36: [v_group(1, 0, 0), v_group(1, 0, 1)],
                39: [k_half(1, 1, 0), k_half(1, 1, 1)],
                40: [kv_load(1, 3)],
                41: [v_half(1, 1, 0), v_half(1, 1, 1)],
                42: [v_group(1, 1, 0), v_group(1, 1, 1)],
                45: [k_half(1, 2, 0), k_half(1, 2, 1)],
                46: [q_load(1, 0)],
                47: [v_half(1, 2, 0), v_half(1, 2, 1)],
                48: [v_group(1, 2, 0), v_group(1, 2, 1)],
                50: [q_load(1, 1)],
                54: [q_load(1, 2)],
                57: [q_half(1, 0, 0), q_half(1, 0, 1)],
                58: [q_load(1, 3)],
            }
            OPS = (8, 10, 12, 14)

            def add(d, key, fn):
                d.setdefault(key, []).append(fn)

            post0 = {}
            for qt in range(2):  # qt0 -> hooks at qt+2, qt1 -> qt+3
                add(post0, (qt + 2, 0), fb(0, qt))
                add(post0, (qt + 2, 6), fcd(0, qt))
                for i in range(4):
                    add(post0, (qt + 2, OPS[i]), outproj(0, qt * 4 + i))
            attention(0, pre0, post0)

            pre1 = {
                2: [q_half(1, 1, 0), q_half(1, 1, 1)],
                4: [k_half(1, 3, 0), k_half(1, 3, 1)],
                6: [v_half(1, 3, 0), v_half(1, 3, 1)],
                8: [v_group(1, 3, 0), v_group(1, 3, 1)],
                18: [q_half(1, 2, 0), q_half(1, 2, 1)],
                34: [q_half(1, 3, 0), q_half(1, 3, 1)],
            }
            post1 = {}
            for qt in (2, 3):  # b0 qt2/qt3 spill into attention(1)
                add(post1, (qt - 2, 0), fb(0, qt))
                add(post1, (qt - 2, 6), fcd(0, qt))
                for i in range(4):
                    add(post1, (qt - 2, OPS[i]), outproj(0, qt * 4 + i))
            for qt in range(2):  # b1 qt0/qt1
                add(post1, (qt + 2, 0), fb(1, qt))
                add(post1, (qt + 2, 6), fcd(1, qt))
                for i in range(4):
                    add(post1, (qt + 2, OPS[i]), outproj(1, qt * 4 + i))
            add(post1, (3, 4), fb(1, 2))
            add(post1, (3, 8), fcd(1, 2))
            attention(1, pre1, post1)
            # tail: the qt3 flush chain leads the DVE queue; outproj tiles
            # follow with copies split between the idle scalar engine + DVE
            fb(1, 3)()
            for i in range(4):
                outproj(1, 8 + i, mode="split")()
            fcd(1, 3)()
            for i in range(4):
                outproj(1, 12 + i, mode="scalar")()

    nc.compile()
    return nc


def make_in_maps(x_q, x_kv, W_qkv, W_out):
    x_q = np.asarray(x_q, dtype=np.float32)
    x_kv = np.asarray(x_kv, dtype=np.float32)
    W_qkv = np.asarray(W_qkv, dtype=np.float32)
    W_out = np.asarray(W_out, dtype=np.float32)

    def chunk_tile(x):
        # [TOK, DIM] -> [n_chunks, 128, KO, PCHUNK] with D = ko*128 + p
        xt = x.reshape(TOK, DIM).T.reshape(KO, 128, TOK // PCHUNK, PCHUNK)
        return np.ascontiguousarray(xt.transpose(2, 1, 0, 3)).astype(BF)

    def w_tile(w):
        # [1024, HD] -> [128, KO, HD] with row = ko*128 + p
        return np.ascontiguousarray(
            w.reshape(KO, 128, HD).transpose(1, 0, 2)
        ).astype(BF)

    xqt = chunk_tile(x_q)
    xkvt = chunk_tile(x_kv)

    in_maps = []
    for c in range(N_CORES):
        cs = slice(c * HD, (c + 1) * HD)
        in_maps.append(
            {
                "xqt": xqt,
                "xkvt": xkvt,
                "wq": w_tile(W_qkv[:, cs]),
                "wk": w_tile(W_qkv[:, 1024:][:, cs]),
                "wv": w_tile(W_qkv[:, 2048:][:, cs]),
                "wout": np.ascontiguousarray(W_out[cs, :]).astype(BF),
            }
        )
    return in_maps


def combine(partials, b_out):
    """Sum the 8 per-core partial projections and add the bias."""
    acc = np.zeros((TOK, DIM), dtype=np.float32)
    for p in partials:
        acc += np.asarray(p, dtype=np.float32)
    acc += np.asarray(b_out, dtype=np.float32)
    return acc.reshape(B, SQ, DIM)


_STATE = {}


def _get_nc():
    if "nc" not in _STATE:
        _STATE["nc"] = build()
    return _STATE["nc"]


def run(x_q, x_kv, W_qkv, W_out, b_out, trace=False):
    nc = _get_nc()
    in_maps = make_in_maps(x_q, x_kv, W_qkv, W_out)
    res = run_bass_kernel_spmd(nc, in_maps, list(range(N_CORES)), trace=trace)
    out = combine([r["out"] for r in res.results], b_out)
    return out, res


def kernel(x_q, x_kv, W_qkv, W_out, b_out):
    out, _ = run(x_q, x_kv, W_qkv, W_out, b_out, trace=False)
    return out
